# revision 14
# baseline (speedup 1.0000x reference)
"""Trainium2 Bass kernel for nn_CharacterLoss: pairwise-cosine BCE loss.

reference:  x = data[indices]; z = cosine-sim(x, x)  [M, M]
            t = token match;  loss = mean(softplus(z) - z * t)

Split the loss algebraically:
  mean(softplus(z) - z*t) = [sum_ij softplus(z_ij) - sum_ij z_ij t_ij] / M^2
The bilinear term collapses exactly:  sum_ij z_ij t_ij = sum_k |g_k|^2,
g_k = sum_{tok_i = k} x_i / |x_i|  -- computed on host in f64 (input prep,
like the gather/normalize).  The device then only needs
sum_ij softplus(z_ij), which is token-independent and symmetric in (i,j),
so each unordered pair is computed once and weighted 2x (diagonal
128x128 subtiles weighted 1x, both orders computed).

On-device math per PSUM quad of z (fp8e4m3 DoubleRow matmuls, D=1024 as
4 k-steps of 256):
  ACT:  s = sigmoid(-z)            (f16 out, one op per 4-bank quad)
  DVE:  grouped products of 8 sigmoids -> pacc  (f16, 2x mode)
  ACT:  tail Ln pass per weight class, fused row-sum accumulator
  sum softplus = -sum ln s;  host combines partials in f64.
Max duplicate-index run is 4, so group-of-8 products are
>= sigma(-1)^4 sigma(-0.17)^4 ~ 2e-4, safely fp16-normal.  Sigmoid and
Ln live in different ACT table sets (1283 ns switch), so Ln tails are
batched across body pairs; Softplus itself has no loadable table in
this toolchain (mybir maps it to pwp 'softplus', absent from act_info;
the 'softplus_and_others' set only carries opaque act1/act2 entries).

Sharding (8 cores): the pairwise matrix is an 8x8 grid of 512x512
blocks.  Core c computes:
  j0: upper triangle of its diagonal block (row-strip w streams cols
      [128w, 512) of its own group) -- 2.5 tile-equivalents,
  j1..j3: blocks (c, c+1..3 mod 8) in full -- 12 tiles, weight 2,
  j4: a half slab of the distance-4 block pair: cores c<4 compute
      z[G_c[0:256], G_{c+4}], cores c>=4 compute z[G_{c-4}[256:512], G_c]
      (both shaped [256, 512]; the w rows are shipped per-core so the
      SPMD program is identical) -- 2 tiles, weight 2.
Total 16.5 tile-equivalents vs the ideal 16.0 (vs 20 for the previous
full-row-strip scheme).  PE streaming is the bottleneck: ~33.8k cols
* 1.13 DR penalty ~ 15.9us predicted at 2.4 GHz (the baseline's
19177 ns = exactly 80 x 241 ns 512-col DR matmuls, i.e. LDWEIGHTS
hides fully behind 512-col streams).

The four w-sharing tiles [j1|j2|j3|triangle] land in one 4-bank
[128, 2048] PSUM quad so a single ACT sigmoid drains all of them (5
sigmoid ops/body instead of 18+); within a quad all groups share each
(w,k) stationary and are k-major interleaved so LDWEIGHTS prefetch
hides behind 512-col matmuls.  Cost-model CoreSim (no_exec) puts the
consume path at ~12.8us/body (ACT 11.2 busy) vs real-PE ~16.3us, so
the body is PE-bound on HW.  Perf notes inherited from the previous
session: GPSIMD tensor_scalar is catastrophically slow on real HW; PE
needs ~3.4us warmup for the HAM clock gate (34 dummy matmuls, outside
the repeat body).  The axon dispatch wall is heavy-tailed (70-100ms,
weather-dependent); only interleaved diff-of-medians A/B is usable.
"""
import os
import sys

sys.path.insert(0, "/opt/trn_rl_repo")

import numpy as np
import ml_dtypes

import concourse.bass as bass
import concourse.mybir as mybir
import concourse.tile as tile
from concourse import bacc
from concourse.bass_utils import run_bass_kernel_spmd

N_CORES = 8
M = 4096
D = 1024
GROUP = M // N_CORES  # 512 rows per block-group
PG = 8  # sigmoid product-group size (max duplicate-run 4 -> min product ~2e-4, fp16-normal)
# per-pair layout: (half_a_xblock, half_a_cols, half_b_xblock, half_b_cols, wa, wb)
# halves a/b occupy PSUM cols [0:512) and [512:512+len_b)
TW = [512, 384, 256, 128]  # triangle widths per w row

_cache = {}
last_result = None  # BassKernelResults of the most recent run (for test.py)


def _build(repeat=1, probe="", sbufs=3, zpbufs=2, swi=False):
    nc = bacc.Bacc("TRN2", target_bir_lowering=False, debug=False)
    dt = mybir.dt
    WCOLS = 768  # 512 own + 256 pair-half rows
    XCOLS = 5 * 512  # own, c+1, c+2, c+3, j4-slab partner
    wT_d = nc.dram_tensor("wT", [128, 4 * 2 * WCOLS], dt.float8e4, kind="ExternalInput").ap()
    xT_d = nc.dram_tensor("xT", [128, 4 * 2 * XCOLS], dt.float8e4, kind="ExternalInput").ap()
    # two partial sums per repeat: [weight-1 cols, weight-2 cols]
    sp_d = nc.dram_tensor("spacc", [128, 2 * repeat], dt.float32, kind="ExternalOutput").ap()

    # quads: one 4-bank PSUM tile per w holding [j1|j2|j3|triangle], each
    # half/quarter an accumulation group sharing the (w,k) stationary; one
    # sigmoid drains the whole quad.  groups: list of (w, xblk, cols, off).
    quads = []
    for w in range(4):
        # triangle strip first within each k-group: if walrus elides the
        # same-weight LDWEIGHTS of the following 512-col matmuls, the one
        # real load per k lands after the previous k's 512-col matmul and
        # stays hidden (neutral if every matmul self-loads).
        quads.append(
            [
                (w, 0, (128 * w, 512), 1536),  # triangle strip, TW cols
                (w, 1, (0, 512), 0),
                (w, 2, (0, 512), 512),
                (w, 3, (0, 512), 1024),
            ]
        )
    quads.append([(4, 4, (0, 512), 0), (5, 4, (0, 512), 512)])  # j4 half-slab

    # pacc column map: weight-1 (diag subtiles) first, then weight-2
    NW1 = 4 * (128 // PG)  # 64
    NW2 = (8448 - 4 * 128) // PG  # 992
    NPACC = NW1 + NW2

    with tile.TileContext(nc) as tc:
        with (
            tc.tile_pool(name="data", bufs=1) as data_pool,
            tc.tile_pool(name="scratch", bufs=sbufs) as scratch,
            tc.tile_pool(name="ps", bufs=zpbufs, space="PSUM") as ps,
        ):
            wall = data_pool.tile([128, 4, 2, WCOLS], dt.float8e4)
            xall = data_pool.tile([128, 4, 2, XCOLS], dt.float8e4)
            wT_r = wT_d.rearrange("p (k j c) -> p k j c", k=4, j=2)
            xT_r = xT_d.rearrange("p (k j c) -> p k j c", k=4, j=2)
            nc.sync.dma_start(out=wall, in_=wT_r)
            for b in range(5):
                nc.sync.dma_start(
                    out=xall[..., b * 512 : (b + 1) * 512],
                    in_=xT_r[..., b * 512 : (b + 1) * 512],
                )

            zbias = data_pool.tile([128, 1], dt.float32)
            nc.vector.memset(zbias, 0.0)
            spacc = data_pool.tile([128, 2 * repeat], dt.float32)

            # PE warmup: the HAM clock gate needs ~3.4us of sustained PE
            # activity to unthrottle 1.2 -> 2.4 GHz.  The warmup PSUM shares
            # the quad pool (all 8 banks belong to the quad rotation).
            dummy = data_pool.tile([128, 128], dt.bfloat16)
            nc.vector.memset(dummy, 0.0)
            dummy_ps = ps.tile([128, 2048], dt.float32, name="zq")
            for _ in range(34):
                nc.tensor.matmul(dummy_ps[:, 0:128], dummy, dummy, start=True, stop=True)

            # second-level product reduce (PG=4, f32 out: min product
            # (2e-4)^4 ~ 3e-15, f32-normal) shrinks the Ln input 4x; Ln
            # tails batch every 4 bodies so the sigmoid<->ln table switch
            # amortizes to 1/2 load per body.
            N2W1 = NW1 // 4  # 16
            N2W2 = NW2 // 4  # 248

            def emit_lvl2(pacc):
                pacc2 = scratch.tile([128, N2W1 + N2W2], dt.float32, name="pacc2", bufs=4)
                for lo, n, col in ((0, NW1, 0), (NW1, NW2, N2W1)):
                    nc.vector.tensor_reduce(
                        out=pacc2[:, col : col + n // 4],
                        in_=pacc[:, lo : lo + n].rearrange("a (g e) -> a g e", e=4),
                        axis=mybir.AxisListType.X,
                        op=mybir.AluOpType.mult,
                    )
                return pacc2

            def emit_ln(r, pacc2):
                # tail ln+accum passes, one per host weight class
                junk1 = scratch.tile([128, N2W1], dt.float32, name="junk1")
                nc.scalar.activation(
                    out=junk1,
                    in_=pacc2[:, :N2W1],
                    func=mybir.ActivationFunctionType.Ln,
                    bias=zbias,
                    scale=1.0,
                    accum_out=spacc[:, 2 * r : 2 * r + 1],
                )
                junk2 = scratch.tile([128, N2W2], dt.float32, name="junk2")
                nc.scalar.activation(
                    out=junk2,
                    in_=pacc2[:, N2W1:],
                    func=mybir.ActivationFunctionType.Ln,
                    bias=zbias,
                    scale=1.0,
                    accum_out=spacc[:, 2 * r + 1 : 2 * r + 2],
                )

            pending = []
            for r in range(repeat):
                pacc = scratch.tile([128, NPACC], dt.float16, name="pacc", bufs=2)
                c1 = 0  # weight-1 cursor
                c2 = NW1  # weight-2 cursor

                for q, groups in enumerate(quads):
                    tri = q < 4
                    width = 1536 + TW[q] if tri else 1024
                    zq = ps.tile([128, 2048], dt.float32, name="zq")
                    # all groups of a tri-quad share the (w,k) stationary --
                    # k-major interleave keeps every LDWEIGHTS prefetch hidden
                    # behind a 512-col matmul.  The j4 quad has two distinct
                    # stationaries: run its groups sequentially.
                    if tri:
                        order = [(g, k) for k in range(4) for g in range(len(groups))]
                    else:
                        order = [(g, k) for g in range(len(groups)) for k in range(4)]
                    pm = (
                        mybir.MatmulPerfMode.DoubleRowSwInterleave
                        if swi
                        else mybir.MatmulPerfMode.DoubleRow
                    )
                    for g, k in order:
                        wt, xt, cols, off = groups[g]
                        nc.tensor.matmul(
                            zq[:, off : off + cols[1] - cols[0]],
                            wall[:, k, :, wt * 128 : (wt + 1) * 128],
                            xall[:, k, :, xt * 512 + cols[0] : xt * 512 + cols[1]],
                            start=(k == 0),
                            stop=(k == 3),
                            perf_mode=pm,
                        )
                    if probe == "pe":
                        continue
                    # s = sigmoid(-z);  softplus(z) = -ln(s)
                    s = scratch.tile([128, width], dt.float16, name="s", bufs=sbufs)
                    nc.scalar.activation(
                        out=s,
                        in_=zq[:, 0:width],
                        func=mybir.ActivationFunctionType.Sigmoid,
                        bias=zbias,
                        scale=-1.0,
                    )

                    def reduce(lo, hi, col):
                        g = (hi - lo) // PG
                        nc.vector.tensor_reduce(
                            out=pacc[:, col : col + g],
                            in_=s[:, lo:hi].rearrange("a (g e) -> a g e", e=PG),
                            axis=mybir.AxisListType.X,
                            op=mybir.AluOpType.mult,
                        )
                        return g

                    if tri:  # [j1|j2|j3 | diag subtile | tri off-diag]
                        c2 += reduce(0, 1536, c2)
                        c1 += reduce(1536, 1664, c1)
                        if width > 1664:
                            c2 += reduce(1664, width, c2)
                    else:  # j4 halves: all weight-2
                        c2 += reduce(0, 1024, c2)

                if probe == "pe":
                    nc.vector.memset(pacc, 0.5)
                else:
                    assert c1 == NW1 and c2 == NPACC, (c1, c2)
                pending.append((r, emit_lvl2(pacc)))
                if len(pending) == 4 or r == repeat - 1:
                    for rr, p2 in pending:
                        emit_ln(rr, p2)
                    pending = []

            nc.sync.dma_start(out=sp_d, in_=spacc)

    nc.compile()
    return nc


def prep_in_maps(data, token_ids, indices):
    data = np.asarray(data, dtype=np.float32)
    token_ids = np.asarray(token_ids)
    indices = np.asarray(indices)

    # host prep: gather, normalize, transpose, quantize
    x = data[indices]  # [M, D] f32
    norms = np.sqrt((x.astype(np.float64) ** 2).sum(-1))
    xh = (x / np.maximum(norms[:, None], 1e-8)).astype(np.float32)
    # DoubleRow fp8 layout: X8[k', p, j, col] = xh[col, k'*256 + 2p + j]
    X8 = np.ascontiguousarray(xh.T.reshape(4, 128, 2, M).astype(ml_dtypes.float8_e4m3))

    in_maps = []
    for c in range(N_CORES):
        own = X8[:, :, :, c * GROUP : (c + 1) * GROUP]
        if c < 4:  # pair-half w rows: own rows [0:256]
            wx = X8[:, :, :, c * GROUP : c * GROUP + 256]
            xj4 = X8[:, :, :, ((c + 4) % 8) * GROUP : ((c + 4) % 8 + 1) * GROUP]
        else:  # pair rows G_{c-4}[256:512]; slab cols = own group
            wx = X8[:, :, :, (c - 4) * GROUP + 256 : (c - 3) * GROUP]
            xj4 = own
        wT = np.concatenate([own, wx], axis=3)
        xT = np.concatenate(
            [X8[:, :, :, ((c + j) % 8) * GROUP : ((c + j) % 8 + 1) * GROUP] for j in range(4)]
            + [xj4],
            axis=3,
        )
        in_maps.append(
            {
                "wT": np.ascontiguousarray(wT.transpose(1, 0, 2, 3)).reshape(128, -1),
                "xT": np.ascontiguousarray(xT.transpose(1, 0, 2, 3)).reshape(128, -1),
            }
        )
    return in_maps


def _zt_term(data, token_ids, indices):
    """sum_ij z_ij t_ij = sum_k |g_k|^2 exactly, in f64 (host input prep)."""
    data = np.asarray(data, dtype=np.float32)
    x = data[np.asarray(indices)].astype(np.float64)
    x /= np.maximum(np.sqrt((x**2).sum(-1))[:, None], 1e-8)
    tok = np.asarray(token_ids)[np.asarray(indices)]
    g = np.zeros((512, D), dtype=np.float64)
    np.add.at(g, tok, x)
    return float((g * g).sum())


def kernel(data, token_ids, indices):
    global last_result
    in_maps = prep_in_maps(data, token_ids, indices)

    if "nc" not in _cache:
        _cache["nc"] = _build()
    nc = _cache["nc"]

    trace = os.environ.get("KERNEL_PROFILE", "") == "1"
    res = run_bass_kernel_spmd(nc, in_maps, list(range(N_CORES)), trace=trace)
    last_result = res

    total = 0.0
    for c in range(N_CORES):
        sp = res.results[c]["spacc"].astype(np.float64)  # [128, 2]
        total += sp[:, 0].sum() + 2.0 * sp[:, 1].sum()
    # spacc holds ln(sigma(-z)) sums = -softplus sums
    loss = (-total - _zt_term(data, token_ids, indices)) / (M * M)
    return np.float32(loss)


# revision 16
# speedup vs baseline: 1.2422x; 1.2422x over previous
"""Trainium2 Bass kernel for nn_CharacterLoss: pairwise-cosine BCE loss.

reference:  x = data[indices]; z = cosine-sim(x, x)  [M, M]
            t = token match;  loss = mean(softplus(z) - z * t)

Split the loss algebraically:
  mean(softplus(z) - z*t) = [sum_ij softplus(z_ij) - sum_ij z_ij t_ij] / M^2
The bilinear term collapses exactly:  sum_ij z_ij t_ij = sum_k |g_k|^2,
g_k = sum_{tok_i = k} x_i / |x_i|  -- computed on host in f64 (input prep,
like the gather/normalize).  The device then only needs
sum_ij softplus(z_ij), which is token-independent and symmetric in (i,j),
so each unordered pair is computed once and weighted 2x (diagonal
128x128 subtiles weighted 1x, both orders computed).

On-device math per PSUM quad of z (fp8e4m3 DoubleRow matmuls, D=1024 as
4 k-steps of 256):
  ACT:  s = sigmoid(-z)            (f16 out, one op per 4-bank quad)
  DVE:  grouped products of 8 sigmoids -> pacc  (f16, 2x mode)
  ACT:  tail Ln pass per weight class, fused row-sum accumulator
  sum softplus = -sum ln s;  host combines partials in f64.
Max duplicate-index run is 4, so group-of-8 products are
>= sigma(-1)^4 sigma(-0.17)^4 ~ 2e-4, safely fp16-normal.  Sigmoid and
Ln live in different ACT table sets (1283 ns switch), so Ln tails are
batched across body pairs; Softplus itself has no loadable table in
this toolchain (mybir maps it to pwp 'softplus', absent from act_info;
the 'softplus_and_others' set only carries opaque act1/act2 entries).

Sharding (8 cores): the pairwise matrix is an 8x8 grid of 512x512
blocks.  Core c computes:
  j0: upper triangle of its diagonal block (row-strip w streams cols
      [128w, 512) of its own group) -- 2.5 tile-equivalents,
  j1..j3: blocks (c, c+1..3 mod 8) in full -- 12 tiles, weight 2,
  j4: a half slab of the distance-4 block pair: cores c<4 compute
      z[G_c[0:256], G_{c+4}], cores c>=4 compute z[G_{c-4}[256:512], G_c]
      (both shaped [256, 512]; the w rows are shipped per-core so the
      SPMD program is identical) -- 2 tiles, weight 2.
Total 16.5 tile-equivalents vs the ideal 16.0 (vs 20 for the previous
full-row-strip scheme).  PE streaming is the bottleneck: ~33.8k cols
* 1.13 DR penalty ~ 15.9us predicted at 2.4 GHz (the baseline's
19177 ns = exactly 80 x 241 ns 512-col DR matmuls, i.e. LDWEIGHTS
hides fully behind 512-col streams).

The four w-sharing tiles [j1|j2|j3|triangle] land in one 4-bank
[128, 2048] PSUM quad so a single ACT sigmoid drains all of them (5
sigmoid ops/body instead of 18+); within a quad all groups share each
(w,k) stationary and are k-major interleaved so LDWEIGHTS prefetch
hides behind 512-col matmuls.  Cost-model CoreSim (no_exec) puts the
consume path at ~12.8us/body (ACT 11.2 busy) vs real-PE ~16.3us, so
the body is PE-bound on HW.  Perf notes inherited from the previous
session: GPSIMD tensor_scalar is catastrophically slow on real HW; PE
needs ~3.4us warmup for the HAM clock gate (34 dummy matmuls, outside
the repeat body).  The axon dispatch wall is heavy-tailed (70-100ms,
weather-dependent); only interleaved diff-of-medians A/B is usable.
"""
import os
import sys

sys.path.insert(0, "/opt/trn_rl_repo")

import numpy as np
import ml_dtypes

import concourse.bass as bass
import concourse.mybir as mybir
import concourse.tile as tile
from concourse import bacc
from concourse.bass_utils import run_bass_kernel_spmd

N_CORES = 8
M = 4096
D = 1024
GROUP = M // N_CORES  # 512 rows per block-group
PG = 8  # sigmoid product-group size (max duplicate-run 4 -> min product ~2e-4, fp16-normal)
# per-pair layout: (half_a_xblock, half_a_cols, half_b_xblock, half_b_cols, wa, wb)
# halves a/b occupy PSUM cols [0:512) and [512:512+len_b)
TW = [512, 384, 256, 128]  # triangle widths per w row

_cache = {}
last_result = None  # BassKernelResults of the most recent run (for test.py)


def _build(repeat=1, probe="", sbufs=3, zpbufs=2, swi=False):
    nc = bacc.Bacc("TRN2", target_bir_lowering=False, debug=False)
    dt = mybir.dt
    WCOLS = 768  # 512 own + 256 pair-half rows
    XCOLS = 5 * 512  # own, c+1, c+2, c+3, j4-slab partner
    wT_d = nc.dram_tensor("wT", [128, 4 * 2 * WCOLS], dt.float8e4, kind="ExternalInput").ap()
    xT_d = nc.dram_tensor("xT", [128, 4 * 2 * XCOLS], dt.float8e4, kind="ExternalInput").ap()
    # one partial sum per repeat: ln(w1 products) + ln(w2 products squared)
    # = -(softplus_w1 + 2 softplus_w2)
    sp_d = nc.dram_tensor("spacc", [128, repeat], dt.float32, kind="ExternalOutput").ap()

    # quads: one 4-bank PSUM tile per w holding [j1|j2|j3|triangle], each
    # half/quarter an accumulation group sharing the (w,k) stationary; one
    # sigmoid drains the whole quad.  groups: list of (w, xblk, cols, off).
    quads = []
    for w in range(4):
        # triangle strip first within each k-group: if walrus elides the
        # same-weight LDWEIGHTS of the following 512-col matmuls, the one
        # real load per k lands after the previous k's 512-col matmul and
        # stays hidden (neutral if every matmul self-loads).
        quads.append(
            [
                (w, 0, (128 * w, 512), 1536),  # triangle strip, TW cols
                (w, 1, (0, 512), 0),
                (w, 2, (0, 512), 512),
                (w, 3, (0, 512), 1024),
            ]
        )
    quads.append([(4, 4, (0, 512), 0), (5, 4, (0, 512), 512)])  # j4 half-slab

    # pacc column map: weight-1 (diag subtiles) first, then weight-2
    NW1 = 4 * (128 // PG)  # 64
    NW2 = (8448 - 4 * 128) // PG  # 992
    NPACC = NW1 + NW2

    with tile.TileContext(nc) as tc:
        with (
            tc.tile_pool(name="data", bufs=1) as data_pool,
            tc.tile_pool(name="scratch", bufs=sbufs) as scratch,
            tc.tile_pool(name="ps", bufs=zpbufs, space="PSUM") as ps,
        ):
            wall = data_pool.tile([128, 4, 2, WCOLS], dt.float8e4)
            xall = data_pool.tile([128, 4, 2, XCOLS], dt.float8e4)
            wT_r = wT_d.rearrange("p (k j c) -> p k j c", k=4, j=2)
            xT_r = xT_d.rearrange("p (k j c) -> p k j c", k=4, j=2)
            nc.sync.dma_start(out=wall, in_=wT_r)
            for b in range(5):
                nc.sync.dma_start(
                    out=xall[..., b * 512 : (b + 1) * 512],
                    in_=xT_r[..., b * 512 : (b + 1) * 512],
                )

            zbias = data_pool.tile([128, 1], dt.float32)
            nc.vector.memset(zbias, 0.0)
            spacc = data_pool.tile([128, repeat], dt.float32)

            # PE warmup: the HAM clock gate needs ~3.4us of sustained PE
            # activity to unthrottle 1.2 -> 2.4 GHz.  The warmup PSUM shares
            # the quad pool (all 8 banks belong to the quad rotation).
            dummy = data_pool.tile([128, 128], dt.bfloat16)
            nc.vector.memset(dummy, 0.0)
            dummy_ps = ps.tile([128, 2048], dt.float32, name="zq")
            for _ in range(34):
                nc.tensor.matmul(dummy_ps[:, 0:128], dummy, dummy, start=True, stop=True)

            # second-level product reduce (PG=4, f32 out: min product
            # (2e-4)^4 ~ 3e-15, f32-normal) shrinks the Ln input 4x; Ln
            # tails batch every 4 bodies so the sigmoid<->ln table switch
            # amortizes to 1/2 load per body.
            N2W1 = NW1 // 4  # 16
            N2W2 = NW2 // 4  # 248

            def emit_lvl2(pacc):
                pacc2 = scratch.tile([128, N2W1 + N2W2], dt.float32, name="pacc2", bufs=8)
                for lo, n, col in ((0, NW1, 0), (NW1, NW2, N2W1)):
                    nc.vector.tensor_reduce(
                        out=pacc2[:, col : col + n // 4],
                        in_=pacc[:, lo : lo + n].rearrange("a (g e) -> a g e", e=4),
                        axis=mybir.AxisListType.X,
                        op=mybir.AluOpType.mult,
                    )
                # fold the host weight 2 into the data: square the w2 region
                nc.vector.tensor_tensor(
                    out=pacc2[:, N2W1:],
                    in0=pacc2[:, N2W1:],
                    in1=pacc2[:, N2W1:],
                    op=mybir.AluOpType.mult,
                )
                return pacc2

            def emit_ln(r, pacc2):
                # single tail ln+accum pass: the w2 region was squared on
                # DVE, so one accumulator yields ln(w1) + 2 ln(w2) directly
                junk1 = scratch.tile([128, N2W1 + N2W2], dt.float32, name="junk1")
                nc.scalar.activation(
                    out=junk1,
                    in_=pacc2,
                    func=mybir.ActivationFunctionType.Ln,
                    bias=zbias,
                    scale=1.0,
                    accum_out=spacc[:, r : r + 1],
                )

            pending = []
            for r in range(repeat):
                pacc = scratch.tile([128, NPACC], dt.float16, name="pacc", bufs=2)
                c1 = 0  # weight-1 cursor
                c2 = NW1  # weight-2 cursor

                for q, groups in enumerate(quads):
                    tri = q < 4
                    width = 1536 + TW[q] if tri else 1024
                    zq = ps.tile([128, 2048], dt.float32, name="zq")
                    # all groups of a tri-quad share the (w,k) stationary --
                    # k-major interleave keeps every LDWEIGHTS prefetch hidden
                    # behind a 512-col matmul.  The j4 quad has two distinct
                    # stationaries: run its groups sequentially.
                    if tri:
                        order = [(g, k) for k in range(4) for g in range(len(groups))]
                    else:
                        order = [(g, k) for g in range(len(groups)) for k in range(4)]
                    pm = (
                        mybir.MatmulPerfMode.DoubleRowSwInterleave
                        if swi
                        else mybir.MatmulPerfMode.DoubleRow
                    )
                    for g, k in order:
                        wt, xt, cols, off = groups[g]
                        nc.tensor.matmul(
                            zq[:, off : off + cols[1] - cols[0]],
                            wall[:, k, :, wt * 128 : (wt + 1) * 128],
                            xall[:, k, :, xt * 512 + cols[0] : xt * 512 + cols[1]],
                            start=(k == 0),
                            stop=(k == 3),
                            perf_mode=pm,
                        )
                    if probe == "pe":
                        continue
                    # s = sigmoid(-z);  softplus(z) = -ln(s)
                    s = scratch.tile([128, width], dt.float16, name="s", bufs=sbufs)
                    nc.scalar.activation(
                        out=s,
                        in_=zq[:, 0:width],
                        func=mybir.ActivationFunctionType.Sigmoid,
                        bias=zbias,
                        scale=-1.0,
                    )

                    def reduce(lo, hi, col):
                        g = (hi - lo) // PG
                        nc.vector.tensor_reduce(
                            out=pacc[:, col : col + g],
                            in_=s[:, lo:hi].rearrange("a (g e) -> a g e", e=PG),
                            axis=mybir.AxisListType.X,
                            op=mybir.AluOpType.mult,
                        )
                        return g

                    if tri:  # [j1|j2|j3 | diag subtile | tri off-diag]
                        c2 += reduce(0, 1536, c2)
                        c1 += reduce(1536, 1664, c1)
                        if width > 1664:
                            c2 += reduce(1664, width, c2)
                    else:  # j4 halves: all weight-2
                        c2 += reduce(0, 1024, c2)

                if probe == "pe":
                    nc.vector.memset(pacc, 0.5)
                else:
                    assert c1 == NW1 and c2 == NPACC, (c1, c2)
                pending.append((r, emit_lvl2(pacc)))
                if len(pending) == 8 or r == repeat - 1:
                    for rr, p2 in pending:
                        emit_ln(rr, p2)
                    pending = []

            nc.sync.dma_start(out=sp_d, in_=spacc)

    nc.compile()
    return nc


def prep_in_maps(data, token_ids, indices):
    data = np.asarray(data, dtype=np.float32)
    token_ids = np.asarray(token_ids)
    indices = np.asarray(indices)

    # host prep: gather, normalize, transpose, quantize
    x = data[indices]  # [M, D] f32
    norms = np.sqrt((x.astype(np.float64) ** 2).sum(-1))
    xh = (x / np.maximum(norms[:, None], 1e-8)).astype(np.float32)
    # DoubleRow fp8 layout: X8[k', p, j, col] = xh[col, k'*256 + 2p + j]
    X8 = np.ascontiguousarray(xh.T.reshape(4, 128, 2, M).astype(ml_dtypes.float8_e4m3))

    in_maps = []
    for c in range(N_CORES):
        own = X8[:, :, :, c * GROUP : (c + 1) * GROUP]
        if c < 4:  # pair-half w rows: own rows [0:256]
            wx = X8[:, :, :, c * GROUP : c * GROUP + 256]
            xj4 = X8[:, :, :, ((c + 4) % 8) * GROUP : ((c + 4) % 8 + 1) * GROUP]
        else:  # pair rows G_{c-4}[256:512]; slab cols = own group
            wx = X8[:, :, :, (c - 4) * GROUP + 256 : (c - 3) * GROUP]
            xj4 = own
        wT = np.concatenate([own, wx], axis=3)
        xT = np.concatenate(
            [X8[:, :, :, ((c + j) % 8) * GROUP : ((c + j) % 8 + 1) * GROUP] for j in range(4)]
            + [xj4],
            axis=3,
        )
        in_maps.append(
            {
                "wT": np.ascontiguousarray(wT.transpose(1, 0, 2, 3)).reshape(128, -1),
                "xT": np.ascontiguousarray(xT.transpose(1, 0, 2, 3)).reshape(128, -1),
            }
        )
    return in_maps


def _zt_term(data, token_ids, indices):
    """sum_ij z_ij t_ij = sum_k |g_k|^2 exactly, in f64 (host input prep)."""
    data = np.asarray(data, dtype=np.float32)
    x = data[np.asarray(indices)].astype(np.float64)
    x /= np.maximum(np.sqrt((x**2).sum(-1))[:, None], 1e-8)
    tok = np.asarray(token_ids)[np.asarray(indices)]
    g = np.zeros((512, D), dtype=np.float64)
    np.add.at(g, tok, x)
    return float((g * g).sum())


def kernel(data, token_ids, indices):
    global last_result
    in_maps = prep_in_maps(data, token_ids, indices)

    if "nc" not in _cache:
        _cache["nc"] = _build()
    nc = _cache["nc"]

    trace = os.environ.get("KERNEL_PROFILE", "") == "1"
    res = run_bass_kernel_spmd(nc, in_maps, list(range(N_CORES)), trace=trace)
    last_result = res

    total = 0.0
    for c in range(N_CORES):
        total += res.results[c]["spacc"].astype(np.float64).sum()
    # spacc holds ln(sigma(-z)) sums = -softplus sums
    loss = (-total - _zt_term(data, token_ids, indices)) / (M * M)
    return np.float32(loss)
